# revision 1
# baseline (speedup 1.0000x reference)
"""Trainium2 Bass kernel for nn_Attention_13073880449373.

Full-batch multi-head attention (B=8, S=1024, C=1024, H=16, D=64) with RoPE,
data-parallel over the batch dim: core b computes batch b end-to-end.

Per-core dataflow (all "T" = channels-on-partitions layout):
  xT (C,S)  --[W_qk as stationary]-->  qkT (2C, S) + per-partition bias (ACT)
  xT (C,S)  --[xT as stationary]  -->  v   (S, C) + bias row via K=1 matmul,
                                       staged to DRAM with interleaved ones col
  RoPE on qkT (DVE; SBUF->SBUF DMA for the rotate-half partition swap)
  scoresT (Sk,Sq) = k'T.T @ q'T per head (K=64)
  pT = exp(0.125 * scoresT)            (ACT, PSUM->SBUF, fp32r out)
  outT (65, Sq) = [v|1].T @ pT         (row 64 = softmax denominators)
  recip = 1/outT[64] (DVE), broadcast over partitions (GPSIMD),
  normalize (DVE) -> attn_outT staged to DRAM
  out (S, C) = attn_outT.T @ W_proj + bias row (K=1 matmul)
All matmul operands live in float32r tiles (full-rate fp32 mode at N=512).
The qk->attention pipeline is interleaved per head-pair to keep PE dense.
"""

import math
import os
from contextlib import ExitStack

import numpy as np

B, S, C = 8, 1024, 1024
H, D = 16, 64
N_CORES = 8
KC = C // 128  # 8 contraction chunks of 128

_CACHE = {}


def _cs_table():
    # Matches reference.rope_cos_sin computed in float32, transposed, with the
    # rotate-half sign folded into the sin half (rows 0-31 negated).
    f = np.float32
    inv = np.exp(np.arange(0, D, 2, dtype=f) * f(-(math.log(10000.0) / D))).astype(f)
    pos = np.arange(S, dtype=f)[:, None]
    ang = (pos * inv[None, :]).astype(f)  # (S, 32)
    ang = np.concatenate([ang, ang], axis=1)  # (S, 64)
    cosT = np.cos(ang).T.astype(f)  # (64, S)
    sinT = np.sin(ang).T.astype(f)
    sign = np.where(np.arange(D) < D // 2, f(-1.0), f(1.0))[:, None].astype(f)
    half = np.concatenate([cosT, sinT * sign], axis=1)  # (64, 2S)
    return np.concatenate([half, half], axis=0).astype(f)  # (128, 2S)


def declare_io(nc):
    from concourse import mybir

    f32 = mybir.dt.float32
    return {
        "xT": nc.dram_tensor("xT", [C, S], f32, kind="ExternalInput").ap(),
        "Wqk": nc.dram_tensor("Wqk", [C + 1, 2 * C], f32, kind="ExternalInput").ap(),
        "Wv": nc.dram_tensor("Wv", [C + 1, C], f32, kind="ExternalInput").ap(),
        "Wp": nc.dram_tensor("Wp", [C + 1, C], f32, kind="ExternalInput").ap(),
        "cs": nc.dram_tensor("cs", [128, 2 * S], f32, kind="ExternalInput").ap(),
        "out": nc.dram_tensor("out", [S, C], f32, kind="ExternalOutput").ap(),
    }


def _emit(tc, io=None):
    from concourse import mybir
    from concourse.bass import ds, ts

    nc = tc.nc
    f32 = mybir.dt.float32
    f32r = mybir.dt.float32r
    AF = mybir.ActivationFunctionType
    MUL = mybir.AluOpType.mult
    ADD = mybir.AluOpType.add

    if io is None:
        io = declare_io(nc)
    xT = io["xT"]
    Wqk = io["Wqk"]
    Wv = io["Wv"]
    Wp = io["Wp"]
    cs = io["cs"]
    out = io["out"]

    with ExitStack() as ctx:
        # ---------------- long-lived consts (right side) ----------------
        kons = ctx.enter_context(tc.tile_pool(name="kons", bufs=1, side="right"))
        ones_sb = kons.tile([1, S], f32, name="ones_sb")
        nc.vector.memset(ones_sb[:], 1.0)
        ones_r = kons.tile([1, S], f32r, name="ones_r")
        nc.vector.tensor_copy(ones_r[:], ones_sb[:])
        # loads emitted below (after xk) to keep the startup queues clear
        cs_t = kons.tile([128, 2 * S], f32, name="cs_t")
        bqk2 = kons.tile([128, 16], f32, name="bqk2")

        dstage = ctx.enter_context(tc.tile_pool(name="dstage", bufs=1, space="DRAM"))
        v_dram = dstage.tile([S, H * 65], f32, name="v_dram")
        aT_dram = dstage.tile([C, S], f32, name="aT_dram")

        mm_ps = ctx.enter_context(tc.tile_pool(name="mm_ps", bufs=2, space="PSUM"))

        # ---------------- activations ----------------
        actx = ctx.enter_context(ExitStack())
        xk_p = actx.enter_context(tc.tile_pool(name="xk", bufs=8))
        xk = []
        for k in range(KC):
            t = xk_p.tile([128, S], f32r, name=f"xk{k}", tag="xk")
            xk.append(t)
        for n in range(2):  # halves so the first matmul chain starts early
            for k in range(KC):
                nc.sync.dma_start(
                    out=xk[k][:, ds(n * 512, 512)],
                    in_=xT[ts(k, 128), ds(n * 512, 512)].bitcast(f32r),
                )
        # RoPE tables + qk bias on the SWDGE/Pool queue (idle this early)
        nc.gpsimd.dma_start(out=cs_t[:], in_=cs[:])
        nc.gpsimd.dma_start(
            out=bqk2[:],
            in_=Wqk[C : C + 1, :].rearrange("o (g p) -> (o p) g", p=128),
        )

        wqk_p = actx.enter_context(tc.tile_pool(name="wqk", bufs=3))
        scr_p = actx.enter_context(tc.tile_pool(name="scr", bufs=2))
        tm_p = actx.enter_context(tc.tile_pool(name="tm", bufs=1))
        qkr_p = actx.enter_context(tc.tile_pool(name="qkr", bufs=6))

        # paired W_qk loads: one DMA per pair -> (128, 8k x (2a x 128c))
        wqk_src = Wqk[0:C, :].rearrange(
            "(k p) (a g c) -> p k g a c", p=128, a=2, g=8
        )

        def qk_pair_weights(pair):
            w = wqk_p.tile([128, 8 * 256], f32r, name=f"wqk{pair}", tag="wqk")
            wv4 = w[:].rearrange("p (k a c) -> p k a c", k=8, a=2)
            for a in range(2):
                nc.scalar.dma_start(
                    out=wv4[:, :, a, :],
                    in_=wqk_src[:, :, pair, a, :].bitcast(f32r),
                )
            return w

        def qk_chunk(pair, a, wts):
            """RoPE'd qkT channel chunk gm = a*8 + pair (a=0: q, a=1: k)."""
            gm = a * 8 + pair
            rr = scr_p.tile([128, 2 * S], f32, name=f"rr{gm}", tag="rr")
            for n in range(2):
                ps = mm_ps.tile([128, 512], f32, name=f"qps{gm}_{n}", tag="mm")
                for k in range(KC):
                    nc.tensor.matmul(
                        ps[:],
                        wts[:, k * 256 + a * 128 : k * 256 + a * 128 + 128],
                        xk[k][:, ds(n * 512, 512)],
                        start=(k == 0),
                        stop=(k == KC - 1),
                    )
                # evacuate + per-channel bias (partition dim here) on DVE
                nc.vector.tensor_scalar_add(
                    rr[:, ds(n * 512, 512)], ps[:], bqk2[:, gm : gm + 1]
                )
            # rotate-half copy (partition swap within each 64-row head)
            for d0, s0 in ((0, 32), (32, 0), (64, 96), (96, 64)):
                nc.gpsimd.dma_start(
                    out=rr[d0 : d0 + 32, S : 2 * S], in_=rr[s0 : s0 + 32, 0:S]
                )
            tm = tm_p.tile([128, 2 * S], f32, name=f"tm{gm}", tag="tm")
            nc.vector.tensor_tensor(tm[:], rr[:], cs_t[:], MUL)
            qt = qkr_p.tile([128, S], f32r, name=f"qkr{gm}", tag="qkr")
            nc.vector.tensor_tensor(qt[:], tm[:, 0:S], tm[:, S : 2 * S], ADD)
            return qt

        # -------- pair 0 qk first (early PE work while weights stream) -----
        w0 = qk_pair_weights(0)
        qt0 = qk_chunk(0, 0, w0)
        kt0 = qk_chunk(0, 1, w0)

        # ---------------- v phase ----------------
        with ExitStack() as vctx:
            wv_p = vctx.enter_context(tc.tile_pool(name="wv", bufs=8))
            bias_v = vctx.enter_context(tc.tile_pool(name="bias_v", bufs=1))
            vst_p = vctx.enter_context(tc.tile_pool(name="vst", bufs=3))
            wv = []
            for k in range(KC):
                t = wv_p.tile([128, C], f32r, name=f"wv{k}", tag="wv")
                nc.scalar.dma_start(out=t[:], in_=Wv[ts(k, 128), :].bitcast(f32r))
                wv.append(t)
            bv = bias_v.tile([1, C], f32r, name="bv")
            nc.sync.dma_start(out=bv[:], in_=Wv[C : C + 1, :].bitcast(f32r))

            for mv in range(S // 128):
                vst = vst_p.tile([128, H * 65], f32, name=f"vst{mv}", tag="vst")
                ones_view = vst[:, 0 : H * 65].rearrange("p (h u) -> p h u", u=65)[
                    :, :, 64:65
                ]
                nc.vector.memset(ones_view, 1.0)
                for n in range(2):
                    ps = mm_ps.tile([128, 512], f32, name=f"vps{mv}_{n}", tag="mm")
                    for k in range(KC + 1):
                        if k < KC:
                            lh = xk[k][:, ts(mv, 128)]
                            rh = wv[k][:, ds(n * 512, 512)]
                        else:
                            lh = ones_r[0:1, ts(mv, 128)]
                            rh = bv[0:1, ds(n * 512, 512)]
                        nc.tensor.matmul(
                            ps[:], lh, rh, start=(k == 0), stop=(k == KC)
                        )
                    ov = vst[:, ds(65 * 8 * n, 65 * 8)].rearrange(
                        "p (h u) -> p h u", u=65
                    )[:, :, 0:64]
                    nc.vector.tensor_copy(ov, ps[:])
                nc.gpsimd.dma_start(out=v_dram[ts(mv, 128), :], in_=vst[:])

        # ---------------- attention pools ----------------
        pT_p = actx.enter_context(tc.tile_pool(name="pT", bufs=7))
        vh_p = actx.enter_context(tc.tile_pool(name="vh", bufs=2))
        rec_p = actx.enter_context(tc.tile_pool(name="rec", bufs=2))
        rb_p = actx.enter_context(tc.tile_pool(name="rb", bufs=2))
        tmo_p = actx.enter_context(tc.tile_pool(name="tmo", bufs=3))
        sc_ps = actx.enter_context(tc.tile_pool(name="sc_ps", bufs=2, space="PSUM"))
        oT_ps = actx.enter_context(tc.tile_pool(name="oT_ps", bufs=4, space="PSUM"))

        vh_src = v_dram[0:S, :].rearrange("(k p) c -> p k c", p=128)

        def attn_begin(pair, qtile, ktile):
            """Head-pair attention, even/odd heads interleaved at the sk level
            so their scores matmuls land on disjoint PE row groups (0-63 vs
            64-127) and run concurrently. PV accumulation trails by 2 sk-steps
            to hide the exp (ACT) latency."""
            heads = (2 * pair, 2 * pair + 1)
            vh = vh_p.tile([128, 8 * 130], f32r, name=f"vh{pair}", tag="vh")
            nc.scalar.dma_start(
                out=vh[:].rearrange("p (k c) -> p k c", c=130),
                in_=vh_src[:, :, 130 * pair : 130 * pair + 130].bitcast(f32r),
            )
            Q = {h: qtile[64 * (h % 2) : 64 * (h % 2) + 64, :] for h in heads}
            Kt = {h: ktile[64 * (h % 2) : 64 * (h % 2) + 64, :] for h in heads}
            oT = {
                h: [
                    oT_ps.tile([65, 512], f32, name=f"oT{h}_{n}", tag="oT")
                    for n in range(2)
                ]
                for h in heads
            }
            pT = {}

            def sc_exp(sk):
                for h in heads:
                    pT[(h, sk)] = pT_p.tile(
                        [128, S], f32r, name=f"pT{h}_{sk}", tag="pT"
                    )
                for n in range(2):
                    for h in heads:  # adjacent MMs on disjoint row groups
                        scps = sc_ps.tile(
                            [128, 512], f32, name=f"sc{h}_{sk}_{n}", tag="sc"
                        )
                        nc.tensor.matmul(
                            scps[:],
                            Kt[h][:, ts(sk, 128)],
                            Q[h][:, ds(n * 512, 512)],
                            start=True,
                            stop=True,
                        )
                        nc.scalar.activation(
                            pT[(h, sk)][:, ds(n * 512, 512)],
                            scps[:],
                            AF.Exp,
                            scale=0.125,
                        )

            def pv(sk):
                for n in range(2):
                    for h in heads:
                        c0 = sk * 130 + 65 * (h % 2)
                        nc.tensor.matmul(
                            oT[h][n][:],
                            vh[:, c0 : c0 + 65],
                            pT[(h, sk)][:, ds(n * 512, 512)],
                            start=(sk == 0),
                            stop=(sk == KC - 1),
                        )

            sc_exp(0)
            sc_exp(1)
            for sk in range(2, KC):
                pv(sk - 2)
                sc_exp(sk)
            return heads, oT, pv

        def attn_finish(state):
            heads, oT, pv = state
            pv(KC - 2)
            pv(KC - 1)
            for h in heads:
                rec = rec_p.tile([1, S], f32, name=f"rec{h}", tag="rec")
                for n in range(2):
                    nc.vector.reciprocal(rec[0:1, ds(n * 512, 512)], oT[h][n][64:65, :])
                rb = rb_p.tile([64, S], f32, name=f"rb{h}", tag="rb")
                nc.gpsimd.partition_broadcast(rb[:], rec[:])
                tmo = tmo_p.tile([64, S], f32, name=f"tmo{h}", tag="tmo")
                for n in range(2):
                    nc.vector.tensor_tensor(
                        tmo[:, ds(n * 512, 512)],
                        oT[h][n][0:64, :],
                        rb[:, ds(n * 512, 512)],
                        MUL,
                    )
                nc.gpsimd.dma_start(out=aT_dram[ds(64 * h, 64), :], in_=tmo[:])

        # software pipeline: next pair's qk chunks are emitted inside the
        # window where this pair's last exps are still draining on ACT.
        qt, kt = qt0, kt0
        for pair in range(H // 2):
            state = attn_begin(pair, qt, kt)
            if pair + 1 < H // 2:
                w = qk_pair_weights(pair + 1)
                qt = qk_chunk(pair + 1, 0, w)
                kt = qk_chunk(pair + 1, 1, w)
            attn_finish(state)

        actx.close()

        # ---------------- output projection ----------------
        with ExitStack() as pctx:
            aT_p = pctx.enter_context(tc.tile_pool(name="aT", bufs=8))
            wp_p = pctx.enter_context(tc.tile_pool(name="wp", bufs=8))
            bias_p = pctx.enter_context(tc.tile_pool(name="bias_p", bufs=1))
            ob_p = pctx.enter_context(tc.tile_pool(name="ob", bufs=3))

            aT = []
            wp = []
            for k in range(KC):
                a = aT_p.tile([128, S], f32r, name=f"aT{k}", tag="aT")
                nc.sync.dma_start(out=a[:], in_=aT_dram[ts(k, 128), :].bitcast(f32r))
                aT.append(a)
                w = wp_p.tile([128, C], f32r, name=f"wp{k}", tag="wp")
                nc.scalar.dma_start(out=w[:], in_=Wp[ts(k, 128), :].bitcast(f32r))
                wp.append(w)
            bp = bias_p.tile([1, C], f32r, name="bp")
            nc.sync.dma_start(out=bp[:], in_=Wp[C : C + 1, :].bitcast(f32r))

            for m in range(S // 128):
                ob = ob_p.tile([128, C], f32, name=f"ob{m}", tag="ob")
                for n in range(2):
                    pp = mm_ps.tile([128, 512], f32, name=f"pp{m}_{n}", tag="mm")
                    for k in range(KC + 1):
                        if k < KC:
                            lh = aT[k][:, ts(m, 128)]
                            rh = wp[k][:, ds(n * 512, 512)]
                        else:
                            lh = ones_r[0:1, ts(m, 128)]
                            rh = bp[0:1, ds(n * 512, 512)]
                        nc.tensor.matmul(
                            pp[:], lh, rh, start=(k == 0), stop=(k == KC)
                        )
                    nc.scalar.activation(ob[:, ds(n * 512, 512)], pp[:], AF.Copy)
                nc.sync.dma_start(out=out[ts(m, 128), :], in_=ob[:])


def build_program():
    """Build + compile the Bass program (cached)."""
    if "nc" in _CACHE:
        return _CACHE["nc"]
    import concourse.tile as tile
    from concourse import bacc

    nc = bacc.Bacc(
        "TRN2", target_bir_lowering=False, debug=False, num_devices=N_CORES
    )
    with tile.TileContext(nc) as tc:
        _emit(tc)
    nc.compile()
    _CACHE["nc"] = nc
    return nc


def host_inputs(x, W_qkv, b_qkv, W_proj, b_proj):
    """Per-core input maps (host-side shard + layout prep)."""
    f = np.float32
    x = np.asarray(x, dtype=f)
    W_qkv = np.asarray(W_qkv, dtype=f)
    b_qkv = np.asarray(b_qkv, dtype=f)
    W_proj = np.asarray(W_proj, dtype=f)
    b_proj = np.asarray(b_proj, dtype=f)
    Wqk = np.concatenate([W_qkv[:, : 2 * C], b_qkv[None, : 2 * C]], axis=0)
    Wv = np.concatenate([W_qkv[:, 2 * C :], b_qkv[None, 2 * C :]], axis=0)
    Wp = np.concatenate([W_proj, b_proj[None, :]], axis=0)
    cs = _cs_table()
    maps = []
    for b in range(B):
        maps.append(
            {
                "xT": np.ascontiguousarray(x[b].T),
                "Wqk": np.ascontiguousarray(Wqk),
                "Wv": np.ascontiguousarray(Wv),
                "Wp": np.ascontiguousarray(Wp),
                "cs": cs,
            }
        )
    return maps


def make_runner():
    """Persistent sharded-jit runner (mirrors bass2jax.run_bass_via_pjrt but
    keeps the compiled executable so repeat kernel() calls don't re-compile)."""
    if "runner" in _CACHE:
        return _CACHE["runner"]
    import jax
    from jax.experimental.shard_map import shard_map
    from jax.sharding import Mesh, PartitionSpec
    from concourse import bass2jax, mybir

    nc = build_program()
    bass2jax.install_neuronx_cc_hook()
    partition_name = nc.partition_id_tensor.name if nc.partition_id_tensor else None

    in_names, out_names, out_avals = [], [], []
    for alloc in nc.m.functions[0].allocations:
        if not isinstance(alloc, mybir.MemoryLocationSet):
            continue
        name = alloc.memorylocations[0].name
        if alloc.kind == "ExternalInput":
            if name != partition_name:
                in_names.append(name)
        elif alloc.kind == "ExternalOutput":
            out_names.append(name)
            out_avals.append(
                jax.core.ShapedArray(
                    tuple(alloc.tensor_shape), mybir.dt.np(alloc.dtype)
                )
            )

    all_in_names = in_names + out_names
    if partition_name is not None:
        all_in_names = all_in_names + [partition_name]

    def _body(*args):
        operands = list(args)
        if partition_name is not None:
            operands.append(bass2jax.partition_id_tensor())
        outs = bass2jax._bass_exec_p.bind(
            *operands,
            out_avals=tuple(out_avals),
            in_names=tuple(all_in_names),
            out_names=tuple(out_names),
            lowering_input_output_aliases=(),
            sim_require_finite=True,
            sim_require_nnan=True,
            nc=nc,
        )
        return tuple(outs)

    devices = jax.devices()[:N_CORES]
    mesh = Mesh(np.asarray(devices), ("core",))
    nin = len(in_names) + len(out_names)
    donate = tuple(range(len(in_names), nin))
    sharded = jax.jit(
        shard_map(
            _body,
            mesh=mesh,
            in_specs=(PartitionSpec("core"),) * nin,
            out_specs=(PartitionSpec("core"),) * len(out_names),
            check_rep=False,
        ),
        donate_argnums=donate,
        keep_unused=True,
    )

    def run(in_maps):
        concat_in = [
            np.concatenate([np.asarray(m[name]) for m in in_maps], axis=0)
            for name in in_names
        ]
        zeros = [
            np.zeros((N_CORES * a.shape[0], *a.shape[1:]), a.dtype)
            for a in out_avals
        ]
        outs = sharded(*concat_in, *zeros)
        return {
            name: np.asarray(outs[i]).reshape(N_CORES, *out_avals[i].shape)
            for i, name in enumerate(out_names)
        }

    _CACHE["runner"] = run
    return run


def _install_neff_cache():
    """Memoize the BIR->NEFF compile so repeat kernel() calls skip the
    multi-minute neuronxcc invocation (pure caching, same artifacts)."""
    if _CACHE.get("neff_cache"):
        return
    import hashlib
    import shutil
    import tempfile

    import concourse.bass2jax as b2j
    import concourse.bass_utils as bu

    cache_dir = os.path.join(tempfile.gettempdir(), "bass_neff_cache")
    os.makedirs(cache_dir, exist_ok=True)
    orig = bu.compile_bir_kernel

    def cached(bir_json, tmpdir, neff_name="file.neff"):
        raw = bir_json if isinstance(bir_json, bytes) else bir_json.encode()
        hit = os.path.join(cache_dir, hashlib.sha256(raw).hexdigest() + ".neff")
        if os.path.exists(hit):
            dst = os.path.join(tmpdir, neff_name)
            shutil.copyfile(hit, dst)
            return dst
        path = orig(bir_json, tmpdir, neff_name)
        try:
            shutil.copyfile(path, hit)
        except OSError:
            pass
        return path

    bu.compile_bir_kernel = cached
    b2j.compile_bir_kernel = cached
    _CACHE["neff_cache"] = True


def kernel(x, W_qkv, b_qkv, W_proj, b_proj):
    from concourse.bass_utils import run_bass_kernel_spmd

    _install_neff_cache()
    nc = build_program()
    in_maps = host_inputs(x, W_qkv, b_qkv, W_proj, b_proj)
    res = run_bass_kernel_spmd(nc, in_maps, list(range(N_CORES)))
    return np.stack([r["out"] for r in res.results], axis=0).astype(np.float32)


if __name__ == "__main__":
    nc = build_program()
    print("program built + compiled OK")



# revision 37
# speedup vs baseline: 140.4347x; 140.4347x over previous
"""Trainium2 Bass kernel for nn_Attention_13073880449373.

Full-batch multi-head attention (B=8, S=1024, C=1024, H=16, D=64) with RoPE,
data-parallel over the batch dim: core b computes batch b end-to-end.

All matmul operands are bf16 (fp32 PSUM accumulation): fp32/f32r moving
operands stream at 2 cycles/column on the PE, bf16 at 1 — this halves PE
time on every matmul. Intermediates (v, attention out) stay SBUF-resident
(no DRAM staging).

Attention core: both heads of a pair land in one 2-bank PSUM scores tile so
a single [128,1024] exp covers the pair (halves ACT instruction count); the
two scores matmuls use disjoint PE row groups (K=64 each) and run
concurrently. Softmax denominators come from a ones-column folded into the
PV stationary (row 64), are staged off PSUM partition 64, batched per pair,
and inverted with one reciprocal_approx_fast; normalization is deferred to
SBUF bf16 tiles (the v bias is added there too — softmax weights sum to 1,
so + b commutes through attention). The steady-state loop pipelines, per
sk-step of window p: filler work (v tiles in window 0, bursty qk-projection
halves for pair p+2 afterwards), PV for pair p, then scores+exp for pair
p+1, keeping PE utilization high (HAM stays at K=8/8) while ACT paces the
windows. The output projection splits k=0..6 partials from the k=7+bias
tail so it overlaps the last pair's normalize chain.
"""

import math
import os
from contextlib import ExitStack

import numpy as np

B, S, C = 8, 1024, 1024
H, D = 16, 64
N_CORES = 8
KC = C // 128  # 8 contraction chunks of 128

_CACHE = {}


def _cs_table():
    # Matches reference.rope_cos_sin computed in float32, transposed, with the
    # rotate-half sign folded into the sin half (rows 0-31 negated).
    import ml_dtypes

    f = np.float32
    inv = np.exp(np.arange(0, D, 2, dtype=f) * f(-(math.log(10000.0) / D))).astype(f)
    pos = np.arange(S, dtype=f)[:, None]
    ang = (pos * inv[None, :]).astype(f)  # (S, 32)
    ang = np.concatenate([ang, ang], axis=1)  # (S, 64)
    cosT = np.cos(ang).T.astype(f)  # (64, S)
    sinT = np.sin(ang).T.astype(f)
    sign = np.where(np.arange(D) < D // 2, f(-1.0), f(1.0))[:, None].astype(f)
    half = np.concatenate([cosT, sinT * sign], axis=1)  # (64, 2S)
    return np.concatenate([half, half], axis=0).astype(ml_dtypes.bfloat16)  # (128, 2S)


def declare_io(nc):
    from concourse import mybir

    f32 = mybir.dt.float32
    bf16 = mybir.dt.bfloat16
    return {
        "xT": nc.dram_tensor("xT", [C, S], bf16, kind="ExternalInput").ap(),
        "Wqk": nc.dram_tensor("Wqk", [C + 1, 2 * C], bf16, kind="ExternalInput").ap(),
        "Wv": nc.dram_tensor("Wv", [C + 1, C], bf16, kind="ExternalInput").ap(),
        "Wp": nc.dram_tensor("Wp", [C + 1, C], bf16, kind="ExternalInput").ap(),
        "cs": nc.dram_tensor("cs", [128, 2 * S], bf16, kind="ExternalInput").ap(),
        "out": nc.dram_tensor("out", [S, C], f32, kind="ExternalOutput").ap(),
    }


def _emit(tc, io=None):
    from concourse import mybir
    from concourse.bass import ds, ts

    nc = tc.nc
    f32 = mybir.dt.float32
    bf16 = mybir.dt.bfloat16
    AF = mybir.ActivationFunctionType
    MUL = mybir.AluOpType.mult
    ADD = mybir.AluOpType.add

    if io is None:
        io = declare_io(nc)
    xT = io["xT"]
    Wqk = io["Wqk"]
    Wv = io["Wv"]
    Wp = io["Wp"]
    cs = io["cs"]
    out = io["out"]

    with ExitStack() as ctx:
        # ---------------- long-lived consts (right side) ----------------
        kons = ctx.enter_context(tc.tile_pool(name="kons", bufs=1, side="right"))
        ones_b = kons.tile([1, S], bf16, name="ones_b")
        nc.vector.memset(ones_b[:], 1.0)
        cs_t = kons.tile([128, 2 * S], bf16, name="cs_t")
        bqk2 = kons.tile([128, 16], f32, name="bqk2")
        bqk2_b = kons.tile([128, 16], bf16, name="bqk2_b")
        # v bias, applied post-softmax (softmax weights sum to 1, so
        # (sum p (v+b))/den == (sum p v)/den + b) — per-head channel columns
        bvT = kons.tile([64, 16], f32, name="bvT")
        bvT_b = kons.tile([64, 16], bf16, name="bvT_b")

        mm_ps = ctx.enter_context(tc.tile_pool(name="mm_ps", bufs=1, space="PSUM"))

        # attention output (proj stationary), persistent across phases
        aT_p = ctx.enter_context(tc.tile_pool(name="aT", bufs=8))
        aT = [aT_p.tile([128, S], bf16, name=f"aT{k}", tag="aT") for k in range(KC)]
        # proj weights, prefetched mid-kernel
        wp_p = ctx.enter_context(tc.tile_pool(name="wp", bufs=8))
        bias_p = ctx.enter_context(tc.tile_pool(name="bias_p", bufs=1))

        # ---------------- activations ----------------
        actx = ctx.enter_context(ExitStack())
        xk_p = actx.enter_context(tc.tile_pool(name="xk", bufs=8))
        xk = []
        for k in range(KC):
            t = xk_p.tile([128, S], bf16, name=f"xk{k}", tag="xk")
            xk.append(t)
        for n in range(2):  # halves so the first matmul chain starts early
            for k in range(KC):
                nc.sync.dma_start(
                    out=xk[k][:, ds(n * 512, 512)],
                    in_=xT[ts(k, 128), ds(n * 512, 512)],
                )
        wqk_p = actx.enter_context(tc.tile_pool(name="wqk", bufs=3))
        scr_p = actx.enter_context(tc.tile_pool(name="scr", bufs=2))
        tm_p = actx.enter_context(tc.tile_pool(name="tm", bufs=1))
        qkr_p = actx.enter_context(tc.tile_pool(name="qkr", bufs=5))
        vst_p = actx.enter_context(tc.tile_pool(name="vst", bufs=8))
        pT_p = actx.enter_context(tc.tile_pool(name="pT", bufs=18))
        tmo_p = actx.enter_context(tc.tile_pool(name="tmo", bufs=4))
        dstg_p = actx.enter_context(tc.tile_pool(name="dstg", bufs=1))
        den_p = actx.enter_context(tc.tile_pool(name="den", bufs=2))
        rec_p = actx.enter_context(tc.tile_pool(name="rec", bufs=2))
        recb_p = actx.enter_context(tc.tile_pool(name="recb", bufs=2))
        rb_p = actx.enter_context(tc.tile_pool(name="rb", bufs=2))
        sc_ps = actx.enter_context(tc.tile_pool(name="sc_ps", bufs=2, space="PSUM"))
        oT_ps = actx.enter_context(tc.tile_pool(name="oT_ps", bufs=3, space="PSUM"))

        # paired W_qk loads: one DMA per (pair, a) -> (128, 8k x 128c)
        wqk_src = Wqk[0:C, :].rearrange("(k p) (a g c) -> p k g a c", p=128, a=2, g=8)

        def qk_pair_weights(pair):
            w = wqk_p.tile([128, 8 * 256], bf16, name=f"wqk{pair}", tag="wqk")
            wv4 = w[:].rearrange("p (k a c) -> p k a c", k=8, a=2)
            for a in range(2):
                nc.sync.dma_start(
                    out=wv4[:, :, a, :],
                    in_=wqk_src[:, :, pair, a, :],
                )
            return w

        def qk_half(pair, a, n, wts, rr):
            """Matmul half n of qkT channel chunk gm = a*8 + pair into rr."""
            gm = a * 8 + pair
            ps = mm_ps.tile([128, 512], f32, name=f"qps{gm}_{n}", tag="mm")
            for k in range(KC):
                nc.tensor.matmul(
                    ps[:],
                    wts[:, k * 256 + a * 128 : k * 256 + a * 128 + 128],
                    xk[k][:, ds(n * 512, 512)],
                    start=(k == 0),
                    stop=(k == KC - 1),
                )
            # evacuate + per-channel bias (partition dim here) on DVE
            nc.vector.tensor_scalar_add(
                rr[:, ds(n * 512, 512)], ps[:], bqk2[:, gm : gm + 1]
            )

        def qk_rope(pair, a, rr):
            """RoPE on a finished rr chunk -> bf16 q/k tile."""
            gm = a * 8 + pair
            # rotate-half copy (partition swap within each 64-row head)
            for d0, s0 in ((0, 32), (32, 0), (64, 96), (96, 64)):
                nc.gpsimd.dma_start(
                    out=rr[d0 : d0 + 32, S : 2 * S], in_=rr[s0 : s0 + 32, 0:S]
                )
            tm = tm_p.tile([128, 2 * S], bf16, name=f"tm{gm}", tag="tm")
            nc.vector.tensor_tensor(tm[:], rr[:], cs_t[:], MUL)
            qt = qkr_p.tile([128, S], bf16, name=f"qkr{gm}", tag="qkr")
            nc.vector.tensor_tensor(qt[:], tm[:, 0:S], tm[:, S : 2 * S], ADD)
            return qt

        def qk_chunk(pair, a, wts):
            rr = scr_p.tile([128, 2 * S], bf16, name=f"rr{a*8+pair}", tag="rr")
            for n in range(2):
                qk_half(pair, a, n, wts, rr)
            return qk_rope(pair, a, rr)

        # -------- qk for pairs 0 and 1 up front (pipeline depth 2) --------
        w0 = qk_pair_weights(0)
        w1 = qk_pair_weights(1)
        # RoPE tables + qk bias land behind the first weight loads
        nc.gpsimd.dma_start(out=cs_t[:], in_=cs[:])
        nc.gpsimd.dma_start(
            out=bqk2_b[:],
            in_=Wqk[C : C + 1, :].rearrange("o (g p) -> (o p) g", p=128),
        )
        nc.vector.tensor_copy(bqk2[:], bqk2_b[:])
        nc.gpsimd.dma_start(
            out=bvT_b[:],
            in_=Wv[C : C + 1, :].rearrange("o (h c) -> (o c) h", c=64),
        )
        nc.vector.tensor_copy(bvT[:], bvT_b[:])

        pT = {}

        def sc_exp(pair, sk):
            """Scores + exp for both heads of a pair at k-chunk sk.

            Both heads' scores land in one 2-bank PSUM tile (h0 cols 0:512,
            h1 cols 512:1024) so a single [128,1024] exp covers the pair —
            half the ACT instruction count. The two matmuls use disjoint PE
            row groups (Kt base partition 0 vs 64) and different banks, so
            they run concurrently."""
            qt, kt = qk[pair]
            heads = (2 * pair, 2 * pair + 1)
            for n in range(2):
                scps = sc_ps.tile(
                    [128, 2 * 512], f32, name=f"sc{pair}_{sk}_{n}", tag="sc"
                )
                for h in heads:
                    i = h % 2
                    nc.tensor.matmul(
                        scps[:, ds(i * 512, 512)],
                        kt[64 * i : 64 * i + 64, ts(sk, 128)],
                        qt[64 * i : 64 * i + 64, ds(n * 512, 512)],
                        start=True,
                        stop=True,
                    )
                p = pT_p.tile([128, 2 * 512], bf16, name=f"pT{pair}_{sk}_{n}", tag="pT")
                pT[(pair, sk, n)] = p
                nc.scalar.activation(p[:], scps[:], AF.Exp, scale=0.125)

        # ---------------- v phase setup (matmuls run inside window 0) -----
        wv_p = actx.enter_context(tc.tile_pool(name="wv", bufs=8))
        wv = []
        for k in range(KC):
            t = wv_p.tile([128, C], bf16, name=f"wv{k}", tag="wv")
            nc.scalar.dma_start(out=t[:], in_=Wv[ts(k, 128), :])
            wv.append(t)

        vst = []

        def v_tile(mv):
            t = vst_p.tile([128, H * 65], bf16, name=f"vst{mv}", tag="vst")
            vst.append(t)
            ones_view = t[:, 0 : H * 65].rearrange("p (h u) -> p h u", u=65)[
                :, :, 64:65
            ]
            nc.vector.memset(ones_view, 1.0)
            for n in range(2):
                ps = mm_ps.tile([128, 512], f32, name=f"vps{mv}_{n}", tag="mm")
                for k in range(KC):
                    nc.tensor.matmul(
                        ps[:],
                        xk[k][:, ts(mv, 128)],
                        wv[k][:, ds(n * 512, 512)],
                        start=(k == 0),
                        stop=(k == KC - 1),
                    )
                ov = t[:, ds(65 * 8 * n, 65 * 8)].rearrange("p (h u) -> p h u", u=65)[
                    :, :, 0:64
                ]
                nc.vector.tensor_copy(ov, ps[:])

        # -------- startup --------
        qt0 = qk_chunk(0, 0, w0)
        kt0 = qk_chunk(0, 1, w0)
        qt1 = qk_chunk(1, 0, w1)
        kt1 = qk_chunk(1, 1, w1)
        qk = {0: (qt0, kt0), 1: (qt1, kt1)}
        for mv in range(3):
            v_tile(mv)
        # exps for pair 0 run on ACT while the v matmuls fill the PE
        for sk in range(KC):
            sc_exp(0, sk)

        # proj weights prefetch (sync queue is quiet from here on)
        wp = []
        for k in range(KC):
            t = wp_p.tile([128, C], bf16, name=f"wpk{k}", tag="wp")
            nc.sync.dma_start(out=t[:], in_=Wp[ts(k, 128), :])
            wp.append(t)
        bp = bias_p.tile([1, C], bf16, name="bp")
        nc.sync.dma_start(out=bp[:], in_=Wp[C : C + 1, :])

        def pv(pair, sk, oT):
            heads = (2 * pair, 2 * pair + 1)
            for n in range(2):
                for h in heads:
                    nc.tensor.matmul(
                        oT[h][n][:],
                        vst[sk][:, 65 * h : 65 * h + 65],
                        pT[(pair, sk, n)][:, ds((h % 2) * 512, 512)],
                        start=(sk == 0),
                        stop=(sk == KC - 1),
                    )
            if sk == KC - 1:
                for s in range(KC):
                    for n in range(2):
                        del pT[(pair, s, n)]

        def finish(pair, oT):
            """Evacuate PV output, invert denominators, normalize into aT."""
            heads = (2 * pair, 2 * pair + 1)
            # denominator rows live on PSUM partition 64; DVE can't shift
            # partitions, so stage them at partition 64 and DMA them down.
            dstg = dstg_p.tile([65, 2 * S], f32, name=f"dstg{pair}", tag="dstg")
            tmo = {}
            # big evacuations first: they free the oT PSUM slots for the
            # next pair's PV; the small denominator rows follow
            for i, h in enumerate(heads):
                t = tmo_p.tile([64, S], bf16, name=f"tmo{h}", tag="tmo")
                tmo[h] = t
                for n in range(2):
                    nc.vector.tensor_copy(
                        t[:, ds(n * 512, 512)], oT[h][n][0:64, :]
                    )
            for i, h in enumerate(heads):
                for n in range(2):
                    nc.vector.tensor_copy(
                        dstg[64:65, ds(i * S + n * 512, 512)], oT[h][n][64:65, :]
                    )
            den = den_p.tile([2, S], f32, name=f"den{pair}", tag="den")
            for i in range(2):
                nc.gpsimd.dma_start(
                    out=den[i : i + 1, :], in_=dstg[64:65, ds(i * S, S)]
                )
            rec = rec_p.tile([2, S], f32, name=f"rec{pair}", tag="rec")
            nc.vector.reciprocal_approx_fast(rec[:], den[:])
            recb = recb_p.tile([2, S], bf16, name=f"recb{pair}", tag="recb")
            nc.vector.tensor_copy(recb[:], rec[:])
            # engines need operands on the same start partition: move row 1
            # of recb down to partition 0 of a scratch row first
            recb1 = recb_p.tile([1, S], bf16, name=f"recb1_{pair}", tag="recb1")
            nc.gpsimd.dma_start(out=recb1[0:1, :], in_=recb[1:2, :])
            rbs = []
            for i, h in enumerate(heads):
                rb = rb_p.tile([64, S], bf16, name=f"rb{h}", tag="rb")
                src = recb[0:1, :] if i == 0 else recb1[0:1, :]
                nc.gpsimd.partition_broadcast(rb[:], src)
                rbs.append(rb)
            for i, h in enumerate(heads):
                nc.vector.tensor_tensor(tmo[h][:], tmo[h][:], rbs[i][:], MUL)
                nc.vector.tensor_scalar_add(tmo[h][:], tmo[h][:], bvT[:, h : h + 1])
                # partition move into the persistent proj stationary tile
                nc.gpsimd.dma_start(
                    out=aT[pair][ds(64 * i, 64), :], in_=tmo[h][:]
                )

        # ---------------- steady-state attention loop ----------------
        # window p, per sk-step: scores+exp(pair+1), PV(pair), filler work
        # (window 0: the v matmul tiles, feeding PV(0) in sk order;
        #  window p>=1: bursty qk-projection halves for pair p+2)
        for pair in range(H // 2):
            heads = (2 * pair, 2 * pair + 1)
            oT = {
                h: [
                    oT_ps.tile([65, 512], f32, name=f"oT{h}_{n}", tag="oT")
                    for n in range(2)
                ]
                for h in heads
            }
            nxt = pair + 2
            if nxt < H // 2:
                w = qk_pair_weights(nxt)
                if pair >= 1:
                    rr_q = scr_p.tile([128, 2 * S], bf16, name=f"rr{nxt}q", tag="rr")
                    rr_k = scr_p.tile([128, 2 * S], bf16, name=f"rr{nxt}k", tag="rr")
            for sk in range(KC):
                # filler first, then PV (always ready), then the ACT-gated
                # scores+exp — keeps ready PE work ahead of slot-blocked MMs
                if pair == 0 and sk >= 3:
                    v_tile(sk)
                if pair >= 1 and nxt < H // 2:
                    if sk == 1:
                        qk_half(nxt, 0, 0, w, rr_q)
                    elif sk == 3:
                        qk_half(nxt, 0, 1, w, rr_q)
                    elif sk == 4:
                        qk[nxt] = (qk_rope(nxt, 0, rr_q), None)
                    elif sk == 5:
                        qk_half(nxt, 1, 0, w, rr_k)
                    elif sk == 7:
                        qk_half(nxt, 1, 1, w, rr_k)
                pv(pair, sk, oT)
                if pair + 1 < H // 2:
                    sc_exp(pair + 1, sk)
            finish(pair, oT)
            if pair == 0 and nxt < H // 2:
                # window 0 was filled by the v tiles; emit qk(2) in one burst
                qk[nxt] = (qk_chunk(nxt, 0, w), qk_chunk(nxt, 1, w))
            elif nxt < H // 2:
                qk[nxt] = (qk[nxt][0], qk_rope(nxt, 1, rr_k))
            del qk[pair]

        actx.close()

        # ---------------- output projection ----------------
        # split per (m, n) group: k=0..6 partials can run while the last
        # pair's normalize chain drains; only k=7 + bias wait on aT[7]
        with ExitStack() as pctx:
            ob_p = pctx.enter_context(tc.tile_pool(name="ob", bufs=3))
            pp_ps = pctx.enter_context(
                tc.tile_pool(name="pp_ps", bufs=7, space="PSUM")
            )
            groups = [(m, n) for m in range(S // 128) for n in range(2)]

            def proj_partial(m, n):
                pp = pp_ps.tile([128, 512], f32, name=f"pp{m}_{n}", tag="pp")
                for k in range(KC - 1):
                    nc.tensor.matmul(
                        pp[:],
                        aT[k][:, ts(m, 128)],
                        wp[k][:, ds(n * 512, 512)],
                        start=(k == 0),
                        stop=False,
                    )
                return pp

            def proj_rest(m, n, pp, ob):
                nc.tensor.matmul(
                    pp[:],
                    aT[KC - 1][:, ts(m, 128)],
                    wp[KC - 1][:, ds(n * 512, 512)],
                    start=False,
                    stop=False,
                )
                nc.tensor.matmul(
                    pp[:],
                    ones_b[0:1, ts(m, 128)],
                    bp[0:1, ds(n * 512, 512)],
                    start=False,
                    stop=True,
                )
                nc.scalar.activation(ob[:, ds(n * 512, 512)], pp[:], AF.Copy)

            pps = {}
            for g in groups[:7]:
                pps[g] = proj_partial(*g)
            obs = {}
            for m, n in groups:
                if (m, n) not in pps:
                    pps[(m, n)] = proj_partial(m, n)
                if m not in obs:
                    obs[m] = ob_p.tile([128, C], f32, name=f"ob{m}", tag="ob")
                proj_rest(m, n, pps[(m, n)], obs[m])
                if n == 1:
                    nc.sync.dma_start(out=out[ts(m, 128), :], in_=obs[m][:])


def build_program():
    """Build + compile the Bass program (cached)."""
    if "nc" in _CACHE:
        return _CACHE["nc"]
    import concourse.tile as tile
    from concourse import bacc

    nc = bacc.Bacc(
        "TRN2", target_bir_lowering=False, debug=False, num_devices=N_CORES
    )
    with tile.TileContext(nc) as tc:
        _emit(tc)
    nc.compile()
    _CACHE["nc"] = nc
    return nc


def host_inputs(x, W_qkv, b_qkv, W_proj, b_proj):
    """Per-core input maps (host-side shard + layout prep, fp32 -> bf16)."""
    import ml_dtypes

    bf = ml_dtypes.bfloat16
    f = np.float32
    x = np.asarray(x, dtype=f)
    W_qkv = np.asarray(W_qkv, dtype=f)
    b_qkv = np.asarray(b_qkv, dtype=f)
    W_proj = np.asarray(W_proj, dtype=f)
    b_proj = np.asarray(b_proj, dtype=f)
    Wqk = np.concatenate([W_qkv[:, : 2 * C], b_qkv[None, : 2 * C]], axis=0).astype(bf)
    Wv = np.concatenate([W_qkv[:, 2 * C :], b_qkv[None, 2 * C :]], axis=0).astype(bf)
    Wp = np.concatenate([W_proj, b_proj[None, :]], axis=0).astype(bf)
    cs = _cs_table()
    maps = []
    for b in range(B):
        maps.append(
            {
                "xT": np.ascontiguousarray(x[b].T).astype(bf),
                "Wqk": np.ascontiguousarray(Wqk),
                "Wv": np.ascontiguousarray(Wv),
                "Wp": np.ascontiguousarray(Wp),
                "cs": cs,
            }
        )
    return maps


def _install_neff_cache():
    """Memoize the BIR->NEFF compile so repeat kernel() calls skip the
    multi-minute neuronxcc invocation (pure caching, same artifacts)."""
    if _CACHE.get("neff_cache"):
        return
    import hashlib
    import shutil
    import tempfile

    import concourse.bass2jax as b2j
    import concourse.bass_utils as bu

    cache_dir = os.path.join(tempfile.gettempdir(), "bass_neff_cache")
    os.makedirs(cache_dir, exist_ok=True)
    orig = bu.compile_bir_kernel

    def cached(bir_json, tmpdir, neff_name="file.neff"):
        raw = bir_json if isinstance(bir_json, bytes) else bir_json.encode()
        hit = os.path.join(cache_dir, hashlib.sha256(raw).hexdigest() + ".neff")
        if os.path.exists(hit):
            dst = os.path.join(tmpdir, neff_name)
            shutil.copyfile(hit, dst)
            return dst
        path = orig(bir_json, tmpdir, neff_name)
        try:
            shutil.copyfile(path, hit)
        except OSError:
            pass
        return path

    bu.compile_bir_kernel = cached
    b2j.compile_bir_kernel = cached
    _CACHE["neff_cache"] = True


def kernel(x, W_qkv, b_qkv, W_proj, b_proj):
    from concourse.bass_utils import run_bass_kernel_spmd

    _install_neff_cache()
    nc = build_program()
    in_maps = host_inputs(x, W_qkv, b_qkv, W_proj, b_proj)
    res = run_bass_kernel_spmd(nc, in_maps, list(range(N_CORES)))
    return np.stack([r["out"] for r in res.results], axis=0).astype(np.float32)


if __name__ == "__main__":
    nc = build_program()
    print("program built + compiled OK")
